# revision 18
# baseline (speedup 1.0000x reference)
"""Trainium2 Bass kernel: multi-head cross-attention block.

Reference computation (per batch b):
    q  = Wq @ x + bq            x = Vx[b] as (C, N=H*W)
    kv = Wkv @ Tx[b] + bkv      split per head into k, v (head h: kv rows
                                256h..256h+128 are k, 256h+128..256h+256 are v)
    attn = softmax(q_h^T k_h * scale) over T
    o_h  = v_h @ attn^T
    out  = Wp @ concat_h(o_h) + bp

Sharding: pure data-parallel over B — 16 batches, 2 per NeuronCore, no
collectives.  All matmuls run as float32r (FP32 storage, FP22 multiply) which
streams at 1 cycle/row on the PE when the moving free dim is >= 256 (and the
free dim is even — ISA restriction).

Softmax layout trick: scores are computed transposed, [t=77 part, n free], so
exp runs directly on that tile and the softmax denominator is produced
*broadcast across all 128 partitions* with a single ones[77,128]^T @ E matmul
(row m of the result = colsum of E for every m).  Normalization is one DVE
reciprocal_approx_fast + one DVE multiply per head.  No max-subtraction is
needed: |scores*scale| <= ~5 for this problem's data scale, exp is safe fp32.

Structure: both batches' k/v are computed up front so the 8 MB Wkv tile pool
can be closed before the n-chunk pools open (SBUF reuse).  DMA emission order
is chosen so the first-needed tensors land first (x chunk0 + Wq for q-proj,
Tx + Wkv for the kv stage, Wp last).

Host-side (free) prep: weights are passed pre-transposed (WqT/WkvT/WpT) so
every DMA is contiguous; Tx is zero-padded to 256 along T (even-N fp32r rule,
full-rate moving dim); biases pre-chunked ([128, 8] per-partition columns)
and the v-bias as a gathered row for the ones-row bias matmul.
"""

import numpy as np

NCORES = 8
B, C, N, T = 16, 1024, 1024, 77
TP = 256             # Tx padded (fp32r matmul needs even free dim; >=256 for full rate)
NH, HD = 8, 128
BPC = B // NCORES        # batches per core
NCHUNK = 512             # n-tile (free dim) size
NCH = N // NCHUNK        # chunks per batch
KC = C // 128            # contraction chunks
SCALE = float(HD) ** -0.5

_CACHE = {}


def _build_module():
    from contextlib import ExitStack

    import concourse.bacc as bacc
    import concourse.mybir as mybir
    import concourse.tile as tile

    f32 = mybir.dt.float32
    f32r = mybir.dt.float32r
    Id = mybir.ActivationFunctionType.Identity
    Exp = mybir.ActivationFunctionType.Exp

    nc = bacc.Bacc("TRN2", debug=False, enable_asserts=False,
                   num_devices=NCORES)

    vx = nc.dram_tensor("vx", [BPC, C, N], f32, kind="ExternalInput").ap()
    tx = nc.dram_tensor("tx", [C, TP], f32, kind="ExternalInput").ap()
    wqT = nc.dram_tensor("wqT", [C, C], f32, kind="ExternalInput").ap()
    wkvK = nc.dram_tensor("wkvK", [C, C], f32, kind="ExternalInput").ap()
    wkvV = nc.dram_tensor("wkvV", [C, C], f32, kind="ExternalInput").ap()
    wpT = nc.dram_tensor("wpT", [C, C], f32, kind="ExternalInput").ap()
    bq2 = nc.dram_tensor("bq2", [128, KC], f32, kind="ExternalInput").ap()
    bk2 = nc.dram_tensor("bk2", [128, NH], f32, kind="ExternalInput").ap()
    bp2 = nc.dram_tensor("bp2", [128, KC], f32, kind="ExternalInput").ap()
    bvr = nc.dram_tensor("bvr", [1, C], f32, kind="ExternalInput").ap()
    onesd = nc.dram_tensor("onesd", [T, 128], f32, kind="ExternalInput").ap()
    out = nc.dram_tensor("out", [BPC, C, N], f32, kind="ExternalOutput").ap()

    def r(ap):
        return ap.bitcast(f32r)

    with tile.TileContext(nc) as tc, ExitStack() as ctx:
        wq_p = ctx.enter_context(tc.tile_pool(name="wq", bufs=1))
        wp_p = ctx.enter_context(tc.tile_pool(name="wp", bufs=1))
        c_p = ctx.enter_context(tc.tile_pool(name="consts", bufs=1))
        kv_p = ctx.enter_context(tc.tile_pool(name="kv", bufs=2))
        x_p = ctx.enter_context(tc.tile_pool(name="x", bufs=12))
        q_p = ctx.enter_context(tc.tile_pool(name="q", bufs=10))
        ps = ctx.enter_context(tc.tile_pool(name="ps", bufs=8, space="PSUM"))

        # ---- DMA emission order = arrival order -----------------------
        # consts (tiny, needed by early evacs), packed Tx + WkvK + WkvV
        # (kv stage), then x00 + Wq (chunk-0 q-proj), then Wp.  The PE
        # executes its stream in program order, so the stream below is
        # kv -> q00 -> chunk loop, matching this arrival order.
        bq_sb = c_p.tile([128, KC], f32, name="bq_sb", tag="bq")
        nc.scalar.dma_start(bq_sb, bq2)
        bk_sb = c_p.tile([128, NH], f32, name="bk_sb", tag="bk")
        nc.scalar.dma_start(bk_sb, bk2)
        bp_sb = c_p.tile([128, KC], f32, name="bp_sb", tag="bp")
        nc.scalar.dma_start(bp_sb, bp2)
        bv_sb = c_p.tile([1, C], f32, name="bv_sb", tag="bv")
        nc.scalar.dma_start(r(bv_sb), r(bvr))
        ones_tm = c_p.tile([T, 128], f32, name="ones_tm", tag="o1")
        nc.scalar.dma_start(r(ones_tm), r(onesd))
        ones_1t = c_p.tile([1, T], f32, name="ones_1t", tag="o2")
        nc.scalar.dma_start(r(ones_1t), r(onesd[0:1, 0:T]))

        txp_t = []
        for cc in range(KC):
            t_ = kv_p.tile([128, TP], f32, name=f"txp{cc}", tag="tx", bufs=KC)
            nc.sync.dma_start(r(t_), r(tx[cc * 128:(cc + 1) * 128, :]))
            txp_t.append(t_)
        wkv_pool = tc.tile_pool(name="wkv", bufs=1)
        wkv_p = wkv_pool.__enter__()
        wkvk_t = []
        for cc in range(KC):
            kt = wkv_p.tile([128, C], f32, name=f"wkvk{cc}", tag=f"wkvk{cc}")
            nc.gpsimd.dma_start(r(kt), r(wkvK[cc * 128:(cc + 1) * 128, :]))
            wkvk_t.append(kt)
        wkvv_t = []
        for cc in range(KC):
            vt_ = wkv_p.tile([128, C], f32, name=f"wkvv{cc}", tag=f"wkvv{cc}")
            eng = nc.sync if cc % 2 == 0 else nc.gpsimd
            eng.dma_start(r(vt_), r(wkvV[cc * 128:(cc + 1) * 128, :]))
            wkvv_t.append(vt_)
        x00_t = []
        for cc in range(KC):
            xt = x_p.tile([128, NCHUNK], f32, name=f"x0_0_{cc}", tag="x")
            nc.sync.dma_start(r(xt), r(vx[0, cc * 128:(cc + 1) * 128,
                                          0:NCHUNK]))
            x00_t.append(xt)
        wq_t = []
        for cc in range(KC):
            wt = wq_p.tile([128, C], f32, name=f"wq{cc}", tag=f"wq{cc}")
            nc.gpsimd.dma_start(r(wt), r(wqT[cc * 128:(cc + 1) * 128, :]))
            wq_t.append(wt)
        wp_t = []
        for cc in range(KC):
            pt = wp_p.tile([128, C], f32, name=f"wp{cc}", tag=f"wp{cc}")
            eng = nc.sync if cc % 2 == 0 else nc.gpsimd
            eng.dma_start(r(pt), r(wpT[cc * 128:(cc + 1) * 128, :]))
            wp_t.append(pt)

        # ---- kv stage: k for BOTH batches in one packed matmul set ----
        # (Tx columns 0:77 = batch 0, 77:154 = batch 1, rest zero pad)
        k_t = [[] for _ in range(BPC)]
        kps_l = [ps.tile([128, TP], f32, name=f"kps{h}", tag="ps")
                 for h in range(NH)]
        for cc in range(KC):
            for h in range(NH):
                lhs = wkvk_t[cc][:, 128 * h:128 * h + 128]
                nc.tensor.matmul(kps_l[h], r(lhs), r(txp_t[cc]),
                                 start=(cc == 0), stop=(cc == KC - 1))
        for h in range(NH):
            for b in range(BPC):
                ksb = kv_p.tile([128, T], f32, name=f"k{b}_{h}", tag="k",
                                bufs=2 * NH)
                nc.scalar.activation(r(ksb), kps_l[h][:, b * T:(b + 1) * T],
                                     Id, bias=bk_sb[:, h:h + 1])
                k_t[b].append(ksb)

        vt_sb = []
        for b in range(BPC):
            vt = kv_p.tile([T, C], f32, name=f"vt{b}", tag="vt", bufs=2)
            vps_l = [ps.tile([T, 512], f32, name=f"vps{b}_{half}", tag="ps")
                     for half in range(2)]
            for cc in range(KC):
                for half in range(2):
                    rhs = wkvv_t[cc][:, 512 * half:512 * half + 512]
                    nc.tensor.matmul(vps_l[half],
                                     r(txp_t[cc][:, b * T:(b + 1) * T]),
                                     r(rhs), start=(cc == 0), stop=False)
            for half in range(2):
                nc.tensor.matmul(vps_l[half], r(ones_1t),
                                 r(bv_sb[:, 512 * half:512 * half + 512]),
                                 start=False, stop=True)
                nc.scalar.copy(r(vt[:, 512 * half:512 * half + 512]),
                               vps_l[half])
            vt_sb.append(vt)

        # Wkv no longer needed — free its SBUF for the chunk pools.
        wkv_pool.__exit__(None, None, None)

        # ---- chunk-0 q-proj (contraction-outer: paces with Wq arrival) --
        q00_ps = [ps.tile([128, NCHUNK], f32, name=f"qps00{d}", tag="ps")
                  for d in range(KC)]
        for cc in range(KC):
            for d in range(KC):
                lhs = wq_t[cc][:, d * 128:(d + 1) * 128]
                nc.tensor.matmul(q00_ps[d], r(lhs), r(x00_t[cc]),
                                 start=(cc == 0), stop=(cc == KC - 1))
        q00_t = []
        for d in range(KC):
            qsb = q_p.tile([128, NCHUNK], f32, name=f"q00{d}", tag="q")
            nc.scalar.activation(r(qsb), q00_ps[d], Id, bias=bq_sb[:, d:d + 1])
            q00_t.append(qsb)

        e_p = ctx.enter_context(tc.tile_pool(name="e", bufs=8))
        ri_p = ctx.enter_context(tc.tile_pool(name="ri", bufs=4))
        on_p = ctx.enter_context(tc.tile_pool(name="on", bufs=10))
        os_p = ctx.enter_context(tc.tile_pool(name="os", bufs=4))

        # ---- n-chunk loop --------------------------------------------
        for b in range(BPC):
            for nco in range(NCH):
                n0 = nco * NCHUNK
                if b == 0 and nco == 0:
                    x_t = x00_t
                else:
                    x_t = []
                    for cc in range(KC):
                        xt = x_p.tile([128, NCHUNK], f32,
                                      name=f"x{b}_{nco}_{cc}", tag="x")
                        nc.sync.dma_start(
                            r(xt),
                            r(vx[b, cc * 128:(cc + 1) * 128, n0:n0 + NCHUNK]))
                        x_t.append(xt)

                if b == 0 and nco == 0:
                    q_t = q00_t
                else:
                    q_t = []
                    for d in range(KC):
                        qps = ps.tile([128, NCHUNK], f32,
                                      name=f"qps{b}{nco}{d}", tag="ps")
                        for cc in range(KC):
                            lhs = wq_t[cc][:, d * 128:(d + 1) * 128]
                            nc.tensor.matmul(qps, r(lhs), r(x_t[cc]),
                                             start=(cc == 0),
                                             stop=(cc == KC - 1))
                        qsb = q_p.tile([128, NCHUNK], f32,
                                       name=f"q{b}{nco}{d}", tag="q")
                        nc.scalar.activation(r(qsb), qps, Id,
                                             bias=bq_sb[:, d:d + 1])
                        q_t.append(qsb)

                on_t = []
                for g in range(NH // 4):
                    hs = range(4 * g, 4 * g + 4)
                    e_l = {}
                    for h in hs:
                        sps = ps.tile([T, NCHUNK], f32,
                                      name=f"sps{b}{nco}{h}", tag="ps")
                        nc.tensor.matmul(sps, r(k_t[b][h]), r(q_t[h]))
                        e_sb = e_p.tile([T, NCHUNK], f32,
                                        name=f"e{b}{nco}{h}", tag="e")
                        nc.scalar.activation(r(e_sb), sps, Exp, scale=SCALE)
                        e_l[h] = e_sb
                    for h in hs:
                        rps = ps.tile([128, NCHUNK], f32,
                                      name=f"rps{b}{nco}{h}", tag="ps")
                        nc.tensor.matmul(rps, r(ones_tm), r(e_l[h]))
                        ri_sb = ri_p.tile([128, NCHUNK], f32,
                                          name=f"ri{b}{nco}{h}", tag="ri")
                        nc.vector.reciprocal_approx_fast(ri_sb, rps)
                        ops_ = ps.tile([128, NCHUNK], f32,
                                       name=f"ops{b}{nco}{h}", tag="ps")
                        nc.tensor.matmul(ops_,
                                         r(vt_sb[b][:, 128 * h:128 * h + 128]),
                                         r(e_l[h]))
                        onrm = on_p.tile([128, NCHUNK], f32,
                                         name=f"on{b}{nco}{h}", tag="on")
                        nc.vector.tensor_mul(r(onrm), ops_, ri_sb)
                        on_t.append(onrm)

                for e in range(KC):
                    fps = ps.tile([128, NCHUNK], f32, name=f"fps{b}{nco}{e}",
                                  tag="ps")
                    for d in range(KC):
                        lhs = wp_t[d][:, e * 128:(e + 1) * 128]
                        nc.tensor.matmul(fps, r(lhs), r(on_t[d]),
                                         start=(d == 0), stop=(d == KC - 1))
                    osb = os_p.tile([128, NCHUNK], f32, name=f"os{b}{nco}{e}",
                                    tag="os")
                    nc.scalar.activation(osb, fps, Id, bias=bp_sb[:, e:e + 1])
                    nc.gpsimd.dma_start(
                        out[b, e * 128:(e + 1) * 128, n0:n0 + NCHUNK], osb)

    nc.compile()
    return nc


def _host_prep(Vx, Tx, Wq, bq, Wkv, bkv, Wp, bp):
    f = np.float32
    Vx3 = np.ascontiguousarray(Vx, dtype=f).reshape(B, C, N)
    TxA = np.asarray(Tx, dtype=f)
    wqT = np.ascontiguousarray(np.asarray(Wq, dtype=f).T)
    Wkv4 = np.asarray(Wkv, dtype=f).reshape(NH, 2, HD, C)
    wkvK_ = np.ascontiguousarray(Wkv4[:, 0].reshape(C, C).T)
    wkvV_ = np.ascontiguousarray(Wkv4[:, 1].reshape(C, C).T)
    wpT = np.ascontiguousarray(np.asarray(Wp, dtype=f).T)
    bq2 = np.ascontiguousarray(np.asarray(bq, dtype=f).reshape(KC, 128).T)
    bkv2 = np.asarray(bkv, dtype=f).reshape(NH, 256)
    bk2 = np.ascontiguousarray(bkv2[:, :128].T)          # [128, NH]
    bvr = np.ascontiguousarray(bkv2[:, 128:].reshape(1, C))
    bp2 = np.ascontiguousarray(np.asarray(bp, dtype=f).reshape(KC, 128).T)

    shared = {"wqT": wqT, "wkvK": wkvK_, "wkvV": wkvV_, "wpT": wpT,
              "bq2": bq2, "bk2": bk2, "bp2": bp2, "bvr": bvr,
              "onesd": np.ones((T, 128), dtype=f)}
    in_maps = []
    for i in range(NCORES):
        m = dict(shared)
        m["vx"] = np.ascontiguousarray(Vx3[i * BPC:(i + 1) * BPC])
        txp = np.zeros((C, TP), dtype=f)
        for bb in range(BPC):
            txp[:, bb * T:(bb + 1) * T] = TxA[i * BPC + bb]
        m["tx"] = txp
        in_maps.append(m)
    return in_maps


def get_module():
    if "nc" not in _CACHE:
        _CACHE["nc"] = _build_module()
    return _CACHE["nc"]


def kernel(**inputs):
    from concourse.bass_utils import run_bass_kernel_spmd

    nc = get_module()
    in_maps = _host_prep(**inputs)
    res = run_bass_kernel_spmd(nc, in_maps, core_ids=list(range(NCORES)))
    outs = [res.results[i]["out"] for i in range(NCORES)]
    full = np.concatenate(outs, axis=0).reshape(B, C, 32, 32)
    return np.ascontiguousarray(full.astype(np.float32))
